# revision 9
# baseline (speedup 1.0000x reference)
"""Grouped (block-diagonal) linear kernel for Trainium2, 8 NeuronCores.

Problem: x [4, 4096, 4096] f32, weight [128, 32, 32], bias [128, 32].
out[b,s,n,o] = sum_i x[b,s,n*32+i] * weight[n,i,o] + bias[n,o], flattened back
to [4, 4096, 4096].

Sharding: the 128 blocks are split across 8 cores (16 blocks = 512 features
per core). Each core reads its own 512-column slice of x and writes the
matching 512-column slice of the output; results are concatenated on host.

Per-core kernel (memory-bound design, ~32 MB in + 32 MB out per core):
  - DMA 1024-token chunks [128p x 4096f] (2 MB per transfer, natural layout).
  - For each 128-token sub-chunk: PE transposes the 4 [128,128] feature
    groups (features -> partitions), ACT copies PSUM->SBUF, then 4 matmuls
    lhsT=xT (stationary) x rhs=block-diag(weights) accumulate into PSUM in
    natural [token, feature] layout. DVE adds bias during the PSUM->SBUF copy.
  - DMA the chunk back out.
"""

import numpy as np

import concourse.bass as bass
import concourse.bacc as bacc
import concourse.mybir as mybir
import concourse.tile as tile

B, S = 4, 4096
IN_F = OUT_F = 4096
NB, IPB, OPB = 128, 32, 32
NCORES = 8
BPC = NB // NCORES            # blocks per core = 16
FPC = BPC * IPB               # features per core = 512
TOK = B * S                   # tokens = 16384
GROUPS = FPC // 128           # 128-feature groups per core = 4
BLOCKS_PER_GROUP = 128 // IPB  # 4

F32 = mybir.dt.float32


def build_nc(
    tok: int = TOK,
    chunk_tok: int = 1024,
    reps: int = 1,
    loop_reps: int = 1,
    use_f32r: bool = False,
):
    """Build the per-core Bass program (SPMD: same program, per-core data).

    reps: python-unrolled repetitions of the whole pass (for timing).
    loop_reps: hardware For_i loop repetitions of the whole pass (for timing
    with constant instruction count).
    use_f32r: stream operands as float32r (same bits as fp32, faster PE
    streaming mode) and run the matmuls as zero-padded pairs with a 256-wide
    moving dim, where f32r hits 1 cycle/row instead of fp32's 4.
    """
    assert tok % chunk_tok == 0 and chunk_tok % 128 == 0
    nchunk = tok // chunk_tok
    sub = chunk_tok // 128     # 128-token sub-chunks per chunk
    XD = mybir.dt.float32r if use_f32r else F32

    nc = bacc.Bacc(
        "TRN2", target_bir_lowering=False, debug=False, num_devices=NCORES
    )
    xs = nc.dram_tensor("xs", [tok, FPC], XD, kind="ExternalInput").ap()
    if use_f32r:
        wpad = nc.dram_tensor(
            "wpad", [GROUPS, 128, 256], XD, kind="ExternalInput"
        ).ap()
    else:
        wbd = nc.dram_tensor("wbd", [GROUPS, 128, 128], F32, kind="ExternalInput").ap()
    bb = nc.dram_tensor("bb", [128, FPC], F32, kind="ExternalInput").ap()
    idn = nc.dram_tensor("idn", [128, 128], XD, kind="ExternalInput").ap()
    out = nc.dram_tensor("out", [tok, FPC], F32, kind="ExternalOutput").ap()

    xs3 = xs.rearrange("(c a p) f -> c p a f", a=sub, p=128)
    out3 = out.rearrange("(c a p) f -> c p a f", a=sub, p=128)

    with tile.TileContext(nc) as tc:
        with (
            tc.tile_pool(name="const", bufs=1) as cpool,
            tc.tile_pool(name="xin", bufs=2) as xpool,
            tc.tile_pool(name="oout", bufs=2) as opool,
            tc.tile_pool(name="xt", bufs=3) as xtpool,
            tc.tile_pool(name="ps", bufs=2, space="PSUM") as pspool,
        ):
            if use_f32r:
                wt = cpool.tile([128, GROUPS * 256], XD)
                nc.sync.dma_start(
                    out=wt[:].rearrange("p (g m) -> p g m", g=GROUPS),
                    in_=wpad.rearrange("g k m -> k g m"),
                )
            else:
                wt = cpool.tile([128, GROUPS * 128], F32)
                nc.sync.dma_start(
                    out=wt[:].rearrange("p (g m) -> p g m", g=GROUPS),
                    in_=wbd.rearrange("g k m -> k g m"),
                )
            bt = cpool.tile([128, FPC], F32)
            nc.sync.dma_start(out=bt[:], in_=bb)
            it = cpool.tile([128, 128], XD)
            nc.sync.dma_start(out=it[:], in_=idn)

            import contextlib

            loop_ctx = (
                tc.For_i(
                    0,
                    loop_reps,
                    1,
                    hint_engines=(mybir.EngineType.PE, mybir.EngineType.Activation),
                )
                if loop_reps > 1
                else contextlib.nullcontext()
            )
            with loop_ctx:
                for _ in range(reps):
                    for c in range(nchunk):
                        x_in = xpool.tile([128, sub * FPC], XD)
                        nc.sync.dma_start(
                            out=x_in[:].rearrange("p (a f) -> p a f", a=sub),
                            in_=xs3[c],
                        )
                        ot = opool.tile([128, sub * FPC], F32)
                        for s in range(sub):
                            xT_ps = pspool.tile([128, FPC], XD)
                            for g in range(GROUPS):
                                nc.tensor.transpose(
                                    xT_ps[:, bass.ts(g, 128)],
                                    x_in[
                                        :, s * FPC + g * 128 : s * FPC + (g + 1) * 128
                                    ],
                                    it[:],
                                )
                            xT_sb = xtpool.tile([128, FPC], XD)
                            nc.scalar.copy(xT_sb[:], xT_ps[:])
                            o_ps = pspool.tile([128, FPC], F32)
                            if use_f32r:
                                for p in range(GROUPS // 2):
                                    for h in range(2):
                                        nc.tensor.matmul(
                                            o_ps[:, bass.ts(p, 256)],
                                            lhsT=xT_sb[:, bass.ts(2 * p + h, 128)],
                                            rhs=wt[:, bass.ts(2 * p + h, 256)],
                                            start=(h == 0),
                                            stop=(h == 1),
                                        )
                            else:
                                for g in range(GROUPS):
                                    nc.tensor.matmul(
                                        o_ps[:, bass.ts(g, 128)],
                                        lhsT=xT_sb[:, bass.ts(g, 128)],
                                        rhs=wt[:, bass.ts(g, 128)],
                                        start=True,
                                        stop=True,
                                    )
                            nc.vector.tensor_add(
                                ot[:, bass.ts(s, FPC)], o_ps[:], bt[:]
                            )
                        nc.scalar.dma_start(
                            out=out3[c],
                            in_=ot[:].rearrange("p (a f) -> p a f", a=sub),
                        )
    nc.compile()
    return nc


def prep_in_maps(x, weight, bias, tok: int = TOK):
    """Split full inputs into 8 per-core input maps (host-side numpy)."""
    x = np.asarray(x, dtype=np.float32).reshape(-1, IN_F)[:tok]
    weight = np.asarray(weight, dtype=np.float32)
    bias = np.asarray(bias, dtype=np.float32)
    ident = np.eye(128, dtype=np.float32)

    in_maps = []
    for m in range(NCORES):
        xs = np.ascontiguousarray(x[:, m * FPC : (m + 1) * FPC])
        wm = weight[m * BPC : (m + 1) * BPC]          # [16, 32, 32]
        wg = np.zeros((GROUPS, 128, 128), np.float32)
        for g in range(GROUPS):
            for a in range(BLOCKS_PER_GROUP):
                wg[g, 32 * a : 32 * a + 32, 32 * a : 32 * a + 32] = wm[
                    BLOCKS_PER_GROUP * g + a
                ]
        # zero-padded pairs for the f32r N=256 matmul path: entry q = 2p+h
        # holds group (2p+h)'s weights in column half h, zeros in the other.
        wp = np.zeros((GROUPS, 128, 256), np.float32)
        for q in range(GROUPS):
            h = q % 2
            wp[q, :, 128 * h : 128 * h + 128] = wg[q]
        bm = bias[m * BPC : (m + 1) * BPC].reshape(FPC)
        bbm = np.ascontiguousarray(np.broadcast_to(bm, (128, FPC)))
        in_maps.append({"xs": xs, "wbd": wg, "wpad": wp, "bb": bbm, "idn": ident})
    return in_maps


def kernel(**inputs) -> np.ndarray:
    from concourse.bass_utils import run_bass_kernel_spmd

    nc = build_nc()
    in_maps = prep_in_maps(inputs["x"], inputs["weight"], inputs["bias"])
    res = run_bass_kernel_spmd(nc, in_maps, core_ids=list(range(NCORES)))
    outs = [res.results[m]["out"] for m in range(NCORES)]
    full = np.concatenate(outs, axis=1)           # [16384, 4096]
    return full.reshape(B, S, OUT_F)
